# revision 18
# baseline (speedup 1.0000x reference)
"""HEX loss kernel for Trainium2 (8 NeuronCores, batch-parallel, raw Bass).

Math: the chain junction-tree potential is rank-1 per clique and each
interior fs[v] is split fs[v]/2 over its two cliques, so the joint
distribution factorizes into independent Bernoullis with
P(y_v=1) = sigmoid(fs[b,v]); hence
    loss = mean_b softplus(-fs[b, labels[b]])

Implementation (per core, 4096 rows): fs is host-cast to bf16 (2 MB)
and streamed via 4 direct SWDGE DMAs at ~300 GB/s (dma_gather was
measured 3-6x slower: ~10 ns/descriptor Q7 gen + ~50-100 GB/s random
256B reads). The label select runs as bf16 big-tile ops: DVE builds
one-hot masks with is_equal(iota256, labt) broadcast APs, multiplies,
and group-reduces to sel = fs[b, lab_b]; two of the four reduces run
on the otherwise-idle GpSimd(Pool) engine. softplus(-sel) =
Ln(1 + Exp(-sel)) runs on ACT (the gen3 natural_log_exp table set has
both Exp and Ln; the table load is issued manually at t=0 so it
overlaps the input DMAs), with accum_out producing [128,1] partials.
A PE matmul against a ones-vector reduces [128,1] -> [1,1] in PSUM so
the final output DMA is a single descriptor -- a [128,1] DRAM store
was measured at 7-10 us of 16-engine completion-sem stagger, vs ~1.8
us for the [1,1] store. Host sums 8 scalars / B.
"""

import numpy as np

B = 32768
V = 256
N_CORES = 8
BL = B // N_CORES          # 4096 rows per core
P = 128
NG = 4                     # stream groups
RPP = 8                    # rows per partition per group
GCOLS = RPP * V            # 2048 bf16 els per partition per group
NT = BL // P               # 32 sel columns total
NLE_TABLE_ID = 6           # natural_log_exp_and_others in gen3 act_info

_CACHE = {}


def _build():
    from contextlib import ExitStack

    import concourse.bass as bass  # noqa
    from concourse import bacc, mybir

    f32 = mybir.dt.float32
    bf16 = mybir.dt.bfloat16
    Alu = mybir.AluOpType
    Act = mybir.ActivationFunctionType

    nc = bacc.Bacc(
        "TRN2",
        target_bir_lowering=False,
        debug=False,
        enable_asserts=False,
        num_devices=N_CORES,
    )

    fs_d = nc.dram_tensor("fs", [BL, V], bf16, kind="ExternalInput").ap()
    cb_d = nc.dram_tensor("cb", [P, NT + V], bf16, kind="ExternalInput").ap()
    ones_d = nc.dram_tensor("ones", [P, 1], f32, kind="ExternalInput").ap()
    out_d = nc.dram_tensor("out", [1, 1], f32, kind="ExternalOutput").ap()

    # row = g*1024 + p*8 + j  ->  group tile [128, (j v)]
    fs_view = fs_d.rearrange("(g p j) v -> g p (j v)", g=NG, p=P, j=RPP)

    with ExitStack() as ctx:
        fs_t = [
            ctx.enter_context(nc.sbuf_tensor(f"fs_t{g}", [P, GCOLS], bf16))
            for g in range(NG)
        ]
        cb = ctx.enter_context(nc.sbuf_tensor("cb_sb", [P, NT + V], bf16))
        onesb = ctx.enter_context(nc.sbuf_tensor("ones_sb", [P, 1], f32))
        mask = ctx.enter_context(nc.sbuf_tensor("mask", [P, NG * GCOLS], bf16))
        prod = ctx.enter_context(nc.sbuf_tensor("prod", [P, NG * GCOLS], bf16))
        sel = ctx.enter_context(nc.sbuf_tensor("sel", [P, NT], f32))
        u32 = ctx.enter_context(nc.sbuf_tensor("u32", [P, NT], f32))
        l32 = ctx.enter_context(nc.sbuf_tensor("l32", [P, NT], f32))
        acc1 = ctx.enter_context(nc.sbuf_tensor("acc1", [P, 1], f32))
        outs = ctx.enter_context(nc.sbuf_tensor("outs", [1, 1], f32))
        po = ctx.enter_context(nc.psum_tensor("po", [1, 1], f32))

        s_c = ctx.enter_context(nc.semaphore("s_c"))
        s_o = ctx.enter_context(nc.semaphore("s_o"))
        s_f = [ctx.enter_context(nc.semaphore(f"s_f{g}")) for g in range(NG)]
        s_p = [ctx.enter_context(nc.semaphore(f"s_p{g}")) for g in range(NG)]
        s_sel = ctx.enter_context(nc.semaphore("s_sel"))
        s_acc = ctx.enter_context(nc.semaphore("s_acc"))
        s_mm = ctx.enter_context(nc.semaphore("s_mm"))
        s_out = ctx.enter_context(nc.semaphore("s_out"))

        labt_ap = cb.ap()[:, 0:NT]          # [128, 32] bf16 labels
        iota_ap = cb.ap()[:, NT : NT + V]   # [128, 256] bf16 iota

        blk = ctx.enter_context(nc.Block())

        @blk.sync
        def _(s_eng):
            s_eng.dma_start(out=cb.ap(), in_=cb_d).then_inc(s_c, 16)
            s_eng.dma_start(out=onesb.ap(), in_=ones_d).then_inc(s_o, 16)

        @blk.gpsimd
        def _(g_eng):
            for g in range(NG):
                g_eng.dma_start(out=fs_t[g].ap(), in_=fs_view[g]).then_inc(
                    s_f[g], 16
                )
            # Pool engine reduces for groups 1 and 3
            for g in (1, 3):
                g_eng.wait_ge(s_p[g], 1)
                g_eng.tensor_reduce(
                    sel.ap()[:, g * RPP : (g + 1) * RPP],
                    prod.ap()[:, g * GCOLS : (g + 1) * GCOLS].rearrange(
                        "p (j v) -> p j v", j=RPP
                    ),
                    axis=mybir.AxisListType.X,
                    op=Alu.add,
                ).then_inc(s_sel, 1)

        @blk.vector
        def _(v_eng):
            v_eng.wait_ge(s_c, 16)
            for g in range(NG):
                v_eng.tensor_tensor(
                    mask.ap()[:, g * GCOLS : (g + 1) * GCOLS].rearrange(
                        "p (j v) -> p j v", j=RPP
                    ),
                    iota_ap.rearrange("p (o v) -> p o v", o=1).broadcast_to(
                        [P, RPP, V]
                    ),
                    labt_ap[:, g * RPP : (g + 1) * RPP]
                    .rearrange("p (j o) -> p j o", o=1)
                    .broadcast_to([P, RPP, V]),
                    Alu.is_equal,
                )
            v_eng.drain()
            for g in range(NG):
                v_eng.wait_ge(s_f[g], 16)
                sl = slice(g * GCOLS, (g + 1) * GCOLS)
                v_eng.tensor_mul(
                    prod.ap()[:, sl], mask.ap()[:, sl], fs_t[g].ap()
                ).then_inc(s_p[g], 1)
            v_eng.drain()
            # DVE reduces for groups 0 and 2
            for g in (0, 2):
                v_eng.tensor_reduce(
                    sel.ap()[:, g * RPP : (g + 1) * RPP],
                    prod.ap()[:, g * GCOLS : (g + 1) * GCOLS].rearrange(
                        "p (j v) -> p j v", j=RPP
                    ),
                    axis=mybir.AxisListType.X,
                    op=Alu.add,
                ).then_inc(s_sel, 1)

        @blk.scalar
        def _(a_eng):
            a_eng.add_instruction(
                mybir.InstLoadActFuncSet(
                    name=nc.get_next_instruction_name(),
                    ins=[],
                    outs=[],
                    act_func_set_id=NLE_TABLE_ID,
                )
            )
            a_eng.wait_ge(s_sel, NG)
            a_eng.activation(u32.ap(), sel.ap(), Act.Exp, scale=-1.0)
            a_eng.drain()
            a_eng.activation(
                l32.ap(), u32.ap(), Act.Ln, bias=1.0, accum_out=acc1.ap()
            ).then_inc(s_acc, 1)
            a_eng.wait_ge(s_mm, 1)
            a_eng.activation(outs.ap(), po.ap(), Act.Identity)
            a_eng.drain()
            a_eng.dma_start(out=out_d, in_=outs.ap()).then_inc(s_out, 16)
            a_eng.wait_ge(s_out, 16)

        @blk.tensor
        def _(t_eng):
            t_eng.wait_ge(s_acc, 1)
            t_eng.wait_ge(s_o, 16)
            t_eng.matmul(
                po.ap(), acc1.ap(), onesb.ap(), start=True, stop=True
            ).then_inc(s_mm, 1)

    nc.compile()
    return nc


def _get_nc():
    if "nc" not in _CACHE:
        _CACHE["nc"] = _build()
    return _CACHE["nc"]


def _shard_inputs(fs, labels):
    import ml_dtypes

    fs = np.asarray(fs, dtype=np.float32)
    labels = np.asarray(labels).astype(np.int64)
    iota256 = np.tile(np.arange(V, dtype=np.float32), (P, 1))  # [128, 256]
    ones = np.ones((P, 1), dtype=np.float32)
    in_maps = []
    for c in range(N_CORES):
        fs_loc = np.ascontiguousarray(fs[c * BL : (c + 1) * BL]).astype(
            ml_dtypes.bfloat16
        )
        lab = labels[c * BL : (c + 1) * BL]
        # labt[p, g*8+j] = lab[g*1024 + p*8 + j]
        labt = (
            lab.reshape(NG, P, RPP).transpose(1, 0, 2).reshape(P, NT)
        ).astype(np.float32)
        cb = np.concatenate([labt, iota256], axis=1).astype(ml_dtypes.bfloat16)
        in_maps.append(
            {
                "fs": fs_loc,
                "cb": np.ascontiguousarray(cb),
                "ones": ones,
            }
        )
    return in_maps


def kernel(fs, labels, _trace=False, _trace_kwargs=None):
    from concourse.bass_utils import run_bass_kernel_spmd

    nc = _get_nc()
    in_maps = _shard_inputs(fs, labels)
    res = run_bass_kernel_spmd(
        nc,
        in_maps,
        core_ids=list(range(N_CORES)),
        trace=_trace,
        **(_trace_kwargs or {}),
    )
    total = np.float64(0.0)
    for c in range(N_CORES):
        total += np.float64(res.results[c]["out"][0, 0])
    loss = total / np.float64(B)
    if _trace:
        return np.float64(loss), res
    return np.asarray(loss, dtype=np.float64)
